# revision 2
# baseline (speedup 1.0000x reference)
"""MoE layer (T=16384, H=1024, F=4096, E=8, top-2) on 8 Trainium2 cores.

Strategy v2 (FFN-dim tensor parallelism — replaces expert-parallel v1):
  - Router runs on host (bit-exact with the reference, same as v1).
  - Every core holds the c-th F/8 slice (FLOC=512) of ALL 8 experts'
    w1/w2 and processes ALL routed (token, expert) pairs, grouped by
    expert in expert-pure chunks of <=512 tokens with EXACT remainder
    chunks — zero capacity padding (v1 padded 4338 -> 4352 per core).
  - Core c computes, for each chunk of expert e:
        hT_c = silu(w1[e,:,Fc]^T @ x_chunk)        [FLOC, N]
        yT_c = w2[e,Fc,:]^T @ hT_c                 [H, N]  (partial sum)
  - Host combine: y = sum_c yT_c, then gate-weighted scatter-add.
  Per-core compute is exactly 32768 pairs x (H*FLOC + FLOC*H) MACs --
  identical across cores, no load imbalance, no padding.
"""

import numpy as np
import ml_dtypes

T, H, F, E, TOPK = 16384, 1024, 4096, 8, 2
P = 128
NC = 8                # cores
FLOC = F // NC        # 512 ffn slice per core
KT = H // P           # 8  k-tiles over H (GEMM1 contraction)
FLT = FLOC // P       # 4  tiles over local F (GEMM2 contraction)
HT = H // P           # 8  output tiles over H
CHUNK = 512

BF16 = ml_dtypes.bfloat16

_module_cache: dict = {}


def _routing(x: np.ndarray, Wg: np.ndarray):
    """Top-2 expert ids and renormalized gates, matching the jax reference.

    The reference receives numpy arrays, so its `x @ Wg` runs through numpy
    BLAS — replicate that exactly (the expert ranking has 1-ulp knife-edge
    ties that flip between BLAS and XLA matmul). softmax/top_k then follow
    the reference's jax ops on CPU.
    """
    logits = x @ Wg  # numpy BLAS fp32, same as reference(**np_inputs)
    try:
        import jax
        import jax.numpy as jnp

        cpu = jax.devices("cpu")[0]
        with jax.default_device(cpu):
            lj = jax.device_put(jnp.asarray(logits), cpu)
            probs = jax.nn.softmax(lj, axis=-1)
            tv, ti = jax.lax.top_k(probs, TOPK)
            rw = tv / jnp.sum(tv, axis=-1, keepdims=True)
        return np.asarray(ti), np.asarray(rw, np.float32)
    except Exception:
        m = logits.max(axis=1, keepdims=True)
        p = np.exp(logits - m)
        p /= p.sum(axis=1, keepdims=True)
        order = np.argsort(-p, axis=1, kind="stable")
        ti = order[:, :TOPK]
        tv = np.take_along_axis(p, ti, axis=1)
        rw = (tv / tv.sum(axis=1, keepdims=True)).astype(np.float32)
        return ti, rw


def _schedule(counts):
    """Expert-pure chunk list [(e, n, off)] with exact remainders.

    Full 512-token chunks run first (expert order — matches the lazy
    weight-load prefetch); remainder chunks are sorted descending so the
    smallest one drains the pipeline at the very end.
    """
    full, rem = [], []
    off = 0
    for e, cnt in enumerate(counts):
        c = int(cnt)
        while c > 0:
            n = min(CHUNK, c)
            (full if n == CHUNK else rem).append((e, n, off))
            off += n
            c -= n
    rem.sort(key=lambda t: -t[1])
    if not full:
        return rem, off
    # Spread remainders through the stream (their per-chunk DMA row cost is
    # fixed, so bunching them starves the output queues); smallest drains
    # last to minimize the pipeline tail.
    last = rem.pop() if rem else None
    out, stride = [], max(1, (len(full) + len(rem)) // (len(rem) + 1) if rem else 1)
    fi = iter(full)
    for r in rem:
        for _ in range(stride):
            nxt = next(fi, None)
            if nxt is not None:
                out.append(nxt)
        out.append(r)
    out.extend(fi)
    if last is not None:
        out.append(last)
    return out, off


def _build_module(counts, repeat: int = 1):
    """Bass/Tile module: yT_partial = sum over chunks of expert FFN slices.

    repeat>1 re-runs the whole chunk loop (same I/O) for differential
    benchmarking.
    """
    import concourse.bass as bass
    import concourse.mybir as mybir
    import concourse.tile as tile
    from concourse import bacc
    from concourse.bass import ts

    dt = mybir.dt
    chunks, ntot = _schedule(counts)

    nc = bacc.Bacc("TRN2", target_bir_lowering=False, debug=False)

    # Partition-major DRAM layouts so each chunk moves with ONE batched 3D
    # DMA (HW per-DMA-instruction overhead dwarfs the modeled descriptor
    # cost when issuing 8 small DMAs instead).
    xT = nc.dram_tensor("xT", (P, KT, ntot), dt.bfloat16, kind="ExternalInput").ap()
    w1p = nc.dram_tensor("w1p", (E, P, KT, FLOC), dt.bfloat16, kind="ExternalInput").ap()
    w2p = nc.dram_tensor("w2p", (E, P, FLT, H), dt.bfloat16, kind="ExternalInput").ap()
    yT = nc.dram_tensor("yT", (P, HT, ntot), dt.bfloat16, kind="ExternalOutput").ap()

    with tile.TileContext(nc) as tc:
        with (
            tc.tile_pool(name="wpool", bufs=1) as wpool,
            tc.tile_pool(name="xpool", bufs=3) as xpool,
            tc.tile_pool(name="hpool", bufs=2) as hpool,
            tc.tile_pool(name="opool", bufs=4) as opool,
            tc.tile_pool(name="spool", bufs=2) as spool,
            tc.tile_pool(name="ps1", bufs=3, space="PSUM") as ps1,
            tc.tile_pool(name="ps2", bufs=5, space="PSUM") as ps2,
        ):
            # Resident weights: all 8 experts' F-slice, 64KB+64KB/partition.
            w1s = [
                wpool.tile([P, KT, FLOC], dt.bfloat16, name=f"w1s{e}")
                for e in range(E)
            ]
            w2s = [
                wpool.tile([P, FLT, H], dt.bfloat16, name=f"w2s{e}")
                for e in range(E)
            ]
            loaded1 = [False] * E
            loaded2 = [False] * E

            def load_w1(e, split=False):
                if loaded1[e]:
                    return
                loaded1[e] = True
                if split:
                    # Finer DMAs for the first expert: the first matmuls can
                    # start as soon as the k=0 slice lands.
                    for k in range(KT):
                        nc.sync.dma_start(out=w1s[e][:, k, :], in_=w1p[e, :, k, :])
                else:
                    nc.sync.dma_start(out=w1s[e][:, :, :], in_=w1p[e, :, :, :])

            def load_w2(e, q=None):
                if loaded2[e]:
                    return
                loaded2[e] = True
                (q or nc.sync).dma_start(out=w2s[e][:, :, :], in_=w2p[e, :, :, :])

            def gemm2(e, n, off, ht, oq, drain=False):
                ot = opool.tile([P, HT, CHUNK], dt.bfloat16, tag="ot")
                for h in range(HT):
                    py = ps2.tile([P, n], dt.float32, tag="py")
                    for fl in range(FLT):
                        nc.tensor.matmul(
                            py[:],
                            lhsT=w2s[e][:, fl, ts(h, P)],
                            rhs=ht[:, fl, :n],
                            start=(fl == 0),
                            stop=(fl == FLT - 1),
                        )
                    # Pool-engine tensor_copy is broken on this runtime
                    # (NRT_EXEC_UNIT_UNRECOVERABLE) — copy on DVE instead.
                    nc.vector.tensor_copy(ot[:, h, :n], py[:])
                if drain:
                    # Final chunk: two half DMAs on both queues so the first
                    # half streams out while the last h-tiles compute.
                    nc.sync.dma_start(
                        out=yT[:, : HT // 2, off : off + n],
                        in_=ot[:, : HT // 2, :n],
                    )
                    nc.scalar.dma_start(
                        out=yT[:, HT // 2 :, off : off + n],
                        in_=ot[:, HT // 2 :, :n],
                    )
                else:
                    oq.dma_start(out=yT[:, :, off : off + n], in_=ot[:, :, :n])

            pending = None
            for idx in range(len(chunks) * repeat):
                e, n, off = chunks[idx % len(chunks)]
                # Weight loads are emitted lazily (first use) so the SP DMA
                # queue stays in demand order; w2 is deferred until after the
                # chunk's x DMA (GEMM2 starts one chunk later anyway). The
                # first x chunk rides the Activation HWDGE queue per-k-tile so
                # it overlaps the first (split) w1 load at t=0.
                xt = xpool.tile([P, KT, CHUNK], dt.bfloat16, tag="xt")
                if idx == 0:
                    # k0 rides SP ahead of the weights (and of the ACT
                    # LoadActFuncSet) so the first matmul fires earliest.
                    nc.sync.dma_start(out=xt[:, 0, :n], in_=xT[:, 0, off : off + n])
                load_w1(e, split=(idx == 0))
                if idx == 0:
                    for k in range(1, KT):
                        nc.scalar.dma_start(
                            out=xt[:, k, :n], in_=xT[:, k, off : off + n]
                        )
                elif idx == 1:
                    # Two half-DMAs: GEMM1(c1) starts on k0-3 while k4-7 land.
                    nc.sync.dma_start(
                        out=xt[:, : KT // 2, :n], in_=xT[:, : KT // 2, off : off + n]
                    )
                    nc.sync.dma_start(
                        out=xt[:, KT // 2 :, :n], in_=xT[:, KT // 2 :, off : off + n]
                    )
                else:
                    nc.sync.dma_start(out=xt[:, :, :n], in_=xT[:, :, off : off + n])
                load_w2(e)
                if idx + 1 < len(chunks):
                    load_w1(chunks[idx + 1][0])
                    load_w2(chunks[idx + 1][0])
                ht = hpool.tile([P, FLT, CHUNK], dt.bfloat16, tag="ht")
                for fl in range(FLT):
                    ph = ps1.tile([P, n], dt.float32, tag="ph")
                    for k in range(KT):
                        nc.tensor.matmul(
                            ph[:],
                            lhsT=w1s[e][:, k, ts(fl, P)],
                            rhs=xt[:, k, :n],
                            start=(k == 0),
                            stop=(k == KT - 1),
                        )
                    # silu(v) = v * sigmoid(v); HW Silu LUT is broken on this
                    # runtime, so compose Sigmoid (ACT) and multiply (DVE).
                    sg = spool.tile([P, CHUNK], dt.float32, tag="sg")
                    nc.scalar.activation(
                        sg[:, :n], ph[:], mybir.ActivationFunctionType.Sigmoid
                    )
                    nc.vector.tensor_mul(ht[:, fl, :n], sg[:, :n], ph[:])
                # GEMM2 of the previous chunk is emitted after GEMM1 of this
                # one so the PE never waits on the silu of its own chunk.
                # Output DMAs alternate between the Activation and SP HWDGE
                # queues to halve per-queue descriptor time.
                if pending is not None:
                    gemm2(*pending, nc.scalar if idx % 2 else nc.sync)
                pending = (e, n, off, ht)
            gemm2(*pending, nc.scalar, drain=True)

    nc.compile()
    return nc


def _get_module(counts, repeat: int = 1):
    key = (tuple(int(c) for c in counts), repeat)
    if key not in _module_cache:
        _module_cache[key] = _build_module(key[0], repeat)
    return _module_cache[key]


def _dispatch(x, ti, rw):
    """Per-expert token lists/gates and the packed transposed input."""
    idx_list, gate_list = [], []
    for e in range(E):
        hit = ti == e
        rows = np.nonzero(hit.any(axis=1))[0]
        g = np.where(hit[rows, 0], rw[rows, 0], rw[rows, 1]).astype(np.float32)
        idx_list.append(rows)
        gate_list.append(g)
    counts = [len(r) for r in idx_list]
    perm = np.concatenate(idx_list)
    xd = x[perm].astype(BF16)                       # [ntot, H]
    xTp = np.ascontiguousarray(xd.reshape(-1, KT, P).transpose(2, 1, 0))
    return idx_list, gate_list, counts, xTp


def kernel(x: np.ndarray, Wg: np.ndarray, w1: np.ndarray, w2: np.ndarray,
           **_unused) -> np.ndarray:
    from concourse.bass_utils import run_bass_kernel_spmd

    x = np.ascontiguousarray(np.asarray(x, np.float32))
    Wg = np.asarray(Wg, np.float32)
    w1 = np.asarray(w1, np.float32)
    w2 = np.asarray(w2, np.float32)
    nt = x.shape[0]

    ti, rw = _routing(x, Wg)
    idx_list, gate_list, counts, xTp = _dispatch(x, ti, rw)

    nc = _get_module(counts)

    in_maps = []
    for c in range(NC):
        sl = slice(c * FLOC, (c + 1) * FLOC)
        in_maps.append(
            {
                "xT": xTp,
                "w1p": np.ascontiguousarray(
                    w1[:, :, sl].astype(BF16).reshape(E, KT, P, FLOC)
                    .transpose(0, 2, 1, 3)
                ),
                "w2p": np.ascontiguousarray(
                    w2[:, sl, :].astype(BF16).reshape(E, FLT, P, H)
                    .transpose(0, 2, 1, 3)
                ),
            }
        )

    res = run_bass_kernel_spmd(nc, in_maps, core_ids=list(range(NC)))

    # Combine partial sums across cores, then gate-weighted scatter-add.
    ysum = np.zeros((H, xTp.shape[2]), np.float32)
    for c in range(NC):
        # yT layout [P, HT, ntot]: global h = ht*128 + p.
        ysum += (
            res.results[c]["yT"].astype(np.float32)
            .transpose(1, 0, 2).reshape(H, -1)
        )
    y = np.zeros((nt, H), np.float32)
    off = 0
    for e in range(E):
        rows = idx_list[e]
        seg = ysum[:, off : off + len(rows)]
        y[rows] += gate_list[e][:, None] * seg.T
        off += len(rows)
    return y


if __name__ == "__main__":
    rng = np.random.default_rng(0)
    xs = rng.standard_normal((T, H), dtype=np.float32)
    Wgs = rng.standard_normal((H, E), dtype=np.float32) / np.sqrt(H)
    w1s = rng.standard_normal((E, H, F), dtype=np.float32) / np.sqrt(H)
    w2s = rng.standard_normal((E, F, H), dtype=np.float32) / np.sqrt(F)
    out = kernel(x=xs, Wg=Wgs, w1=w1s, w2=w2s)
    print(out.shape, out.dtype)
